# revision 13
# baseline (speedup 1.0000x reference)
"""Ring-attention (context-parallel) kernel for 8 TRN2 NeuronCores.

Problem: x_q [1,2048,2048], x_kv [1,8192,2048], GQA attention (16 q heads,
4 kv heads, D=128) where q occupies global positions 6144..8191 of the
8192-long key sequence (causal on the last 2048 block, full attention on
the first 6144 keys), followed by an output projection.

Strategy (sequence/context parallel):
  - q rows split into 16 strips of 128; core c owns strips {c, 15-c}
    (folded pairing balances the causal wedge).
  - x_kv sequence-sharded 8 x 1024 rows; each core projects its local
    K/V shard. AGa gathers [K^T_0 | V_0] (what group 0 needs), AGb the
    V of groups 1-3, AG1..3 the per-group K^T -- all triggered from
    phase A so attention group g's data always lands early.
  - Host-side arrays are packed partition-major so every DMA moves fat
    contiguous per-partition lines.
  - Attention: S matmuls are N=512 (4 heads x 128 q per strip) per key
    chunk; fully-masked strip halves skipped (kc>=56). PV trails S/exp
    by two chunks to ride out ACT latency.
  - Softmax denominator: bf16 chunk accumulation on DVE + a PE
    ones-matmul (column-sum broadcast); norm runs right at the group
    boundary. (A gpsimd partition_all_reduce here would block the
    in-order DVE queue ~12us per group.)
  - Q-proj heads 4-15 and the previous group's o_proj are emitted as
    fine-grained filler (a few matmuls per key chunk) inside the rank
    loops, so the PE never idles while ACT exps stream and group spans
    stay above the ACT floor.
  - o_proj uses stationary=ao q-block, rhs=Wo rows (N=512): 32 matmuls
    per group instead of 64 LDW-bound N=256 ones. Output is [q, hid].
"""

import numpy as np
import ml_dtypes

import concourse.bass as bass
import concourse.mybir as mybir
import concourse.tile as tile
from concourse import bacc, bass_isa, bass_utils

BF16 = ml_dtypes.bfloat16
F32 = mybir.dt.float32
BF = mybir.dt.bfloat16

N_CORES = 8
H = 16          # query heads
HKV = 4         # kv heads
D = 128         # head dim
HID = H * D     # 2048
SL = 2048       # q rows (global)
SKV = 8192      # kv rows (global)
QS = 256        # q rows per core (2 strips of 128)
LKV = SKV // N_CORES   # 1024 local kv rows
HC = HID // 128        # 16 hid chunks
KC = SKV // 128        # 64 key chunks
RANK_OFF = SKV - SL    # 6144: global position of q row 0
BND = RANK_OFF // 128  # 48: first key chunk needing a causal mask
SCALE = 1.0 / float(np.sqrt(D))

KREG = 128 * LKV       # 131072 elems: K^T region of bncA
VREG = 128 * LKV       # V_0 region of bncA (8 lc x 128 k x 128 d)

_CACHE = {}


def _build():
    nc = bacc.Bacc("TRN2", target_bir_lowering=False, debug=False,
                   num_devices=N_CORES)

    xqT = nc.dram_tensor("xqT", [128, HC, QS], BF, kind="ExternalInput")
    xkvT = nc.dram_tensor("xkvT", [128, HC, LKV], BF, kind="ExternalInput")
    wqP = nc.dram_tensor("wqP", [128, H, HC * 128], BF, kind="ExternalInput")
    wkT = nc.dram_tensor("wkT", [128, HC, HKV * D], BF, kind="ExternalInput")
    wvT = nc.dram_tensor("wvT", [128, HC, HKV * D], BF, kind="ExternalInput")
    woP = nc.dram_tensor("woP", [HKV, 128, 4 * HID], BF, kind="ExternalInput")
    # boundary causal masks, (strip,head,q) col order: j<8 -> strip0 mask,
    # j>=8 -> strip1 mask, each [128 keys, 4 heads x 128 q]
    maskD = nc.dram_tensor("mask", [128, 16, 512], BF, kind="ExternalInput")
    outT = nc.dram_tensor("outT", [QS, HID], F32, kind="ExternalOutput")

    with tile.TileContext(nc) as tc:
        _body(nc, tc, xqT, xkvT, wqP, wkT, wvT, woP, maskD, outT)
    nc.compile()
    return nc


def _body(nc, tc, xqT, xkvT, wqP, wkT, wvT, woP, maskD, outT):
    from contextlib import ExitStack
    ctx = ExitStack()
    with ctx:
        persist = ctx.enter_context(tc.tile_pool(name="persist", bufs=1))
        dram = ctx.enter_context(tc.tile_pool(name="dram", bufs=1, space="DRAM"))

        # resident tiles
        qt_sb = persist.tile([128, 2, H, 128], BF)    # Q^T [D, strip, head, q]
        ao_sb = persist.tile([128, HKV, 4, QS], BF)   # normalized O^T per g
        out_acc = persist.tile([128, 2, HID], F32)    # [q-in-strip, s, hid]
        ones_sb = persist.tile([128, 128], BF)
        nc.vector.memset(ones_sb[:], 1.0)

        # bounce buffers
        bncA = dram.tile([KREG + VREG], BF, name="bncA", uniquify=False)
        bncB = dram.tile([3 * VREG], BF, name="bncB", uniquify=False)
        bncK = [dram.tile([256, 512], BF, name=f"bnck{g}", uniquify=False)
                for g in range(1, HKV)]
        rg = [list(range(N_CORES))]
        gathA = dram.tile([N_CORES, KREG + VREG], BF, addr_space="Shared",
                          name="gathA", uniquify=False)
        gathB = dram.tile([N_CORES, 3 * VREG], BF, addr_space="Shared",
                          name="gathB", uniquify=False)
        gathK = [dram.tile([N_CORES, 256, 512], BF, addr_space="Shared",
                           name=f"gathk{g}", uniquify=False)
                 for g in range(1, HKV)]

        # ---------------- Phase A: local K/V projection -------------------
        with (
            tc.tile_pool(name="kva", bufs=1) as kva,
            tc.tile_pool(name="kvps", bufs=2, space="PSUM") as kvps,
            tc.tile_pool(name="vps", bufs=2, space="PSUM") as vps,
        ):
            xkv_sb = kva.tile([128, HC, LKV], BF)
            wk_sb = kva.tile([128, HC, HKV * D], BF)
            wv_sb = kva.tile([128, HC, HKV * D], BF)
            # chunked 4-hc loads so the first K0 matmuls start ~3us in
            for c4 in range(4):
                hs = slice(4 * c4, 4 * c4 + 4)
                nc.sync.dma_start(xkv_sb[:, hs, :], xkvT.ap()[:, hs, :])
                nc.sync.dma_start(wk_sb[:, hs, :], wkT.ap()[:, hs, :])
                nc.sync.dma_start(wv_sb[:, hs, :], wvT.ap()[:, hs, :])
            xq_sb = persist.tile([128, HC, QS], BF)
            nc.sync.dma_start(xq_sb[:], xqT.ap())
            mask_sb = persist.tile([128, 16, 512], BF)
            nc.sync.dma_start(mask_sb[:], maskD.ap())
            wq_sb = persist.tile([128, H, HC * 128], BF)
            nc.sync.dma_start(wq_sb[:], wqP.ap())

            def k_proj(g):
                ps = kvps.tile([128, LKV], F32, tag="kt")
                for hc in range(HC):
                    lhsT = wk_sb[:, hc, g * D:(g + 1) * D]
                    for nn in range(0, LKV, 512):
                        nc.tensor.matmul(
                            ps[:, nn:nn + 512], lhsT,
                            xkv_sb[:, hc, nn:nn + 512],
                            start=(hc == 0), stop=(hc == HC - 1))
                # locs live in persist: a phase-A pool close must not wait
                # for the trailing bounce-write DMAs (13us on one queue)
                kt_loc = persist.tile([128, LKV], BF, tag="ktloc", bufs=2,
                                      name=f"ktloc{g}")
                nc.vector.tensor_copy(kt_loc[:], ps[:])
                if g == 0:
                    dst = bncA[0:KREG].rearrange("(p c) -> p c", p=128)
                else:
                    dst = bncK[g - 1][0:256, :].rearrange(
                        "(p a) c -> p (a c)", p=128)
                nc.gpsimd.dma_start(dst, kt_loc[:])

            prev_cc = [None]

            def ag(src, dst):
                # chain AGs with explicit deps: collectives are engine-less
                # instructions and the scheduler otherwise linearizes them
                # in arbitrary NEFF order (NRT executes them in NEFF order,
                # so a late AGa starves attention group 0)
                cc = nc.gpsimd.collective_compute(
                    "AllGather", mybir.AluOpType.bypass, replica_groups=rg,
                    ins=[src.opt()], outs=[dst.opt()])
                if prev_cc[0] is not None:
                    bass._add_dep_helper(cc.ins, prev_cc[0].ins, sync=True,
                                         reason="serialize AG order")
                prev_cc[0] = cc

            k_proj(0)
            # V (N=512 across all 4 groups); V_0 into bncA, V_1..3 into bncB
            for lc in range(LKV // 128):
                ps = vps.tile([128, HKV * D], F32, tag="v")
                for hc in range(HC):
                    nc.tensor.matmul(
                        ps[:], xkv_sb[:, hc, lc * 128:(lc + 1) * 128],
                        wv_sb[:, hc, :],
                        start=(hc == 0), stop=(hc == HC - 1))
                v_loc = persist.tile([128, HKV * D], BF, tag="vloc", bufs=4,
                                     name=f"vloc{lc}")
                nc.vector.tensor_copy(v_loc[:], ps[:])
                # k-major V regions: writes are thin (256B/partition) but
                # spread across queues; reads become one fat DMA per rank
                nc.gpsimd.dma_start(
                    bncA[KREG:KREG + VREG]
                    .rearrange("(p a d) -> p a d", p=128, d=D)[:, lc, :],
                    v_loc[:, 0:128])
                for gp in range(3):
                    nc.sync.dma_start(
                        bncB[gp * VREG:(gp + 1) * VREG]
                        .rearrange("(p a d) -> p a d", p=128, d=D)[:, lc, :],
                        v_loc[:, (gp + 1) * 128:(gp + 2) * 128])
            ag(bncA, gathA)
            ag(bncB, gathB)
            for g in range(1, HKV):
                k_proj(g)
                ag(bncK[g - 1], gathK[g - 1])

        # ---------------- Phase D: attention + fused projections ----------
        with (
            tc.tile_pool(name="kvstream", bufs=4) as kvstream,
            tc.tile_pool(name="attw", bufs=3) as attw,
            tc.tile_pool(name="accp", bufs=1) as accp,
            tc.tile_pool(name="wop", bufs=2) as wop,
            tc.tile_pool(name="stps", bufs=2, space="PSUM") as stps,
            tc.tile_pool(name="otps", bufs=1, space="PSUM") as otps,
            tc.tile_pool(name="fps", bufs=2, space="PSUM") as fps,
        ):
            wos = {}
            ot_pss = {}

            def q_head(h):
                # project one q head; fp-ring PSUM, yields every 2 matmuls
                ps = fps.tile([128, 512], F32, tag="fp", name=f"qp{h}")
                for hc2 in range(8):
                    for hc in (2 * hc2, 2 * hc2 + 1):
                        nc.tensor.matmul(
                            ps[:, 0:QS], wq_sb[:, h, hc * 128:(hc + 1) * 128],
                            xq_sb[:, hc, :],
                            start=(hc == 0), stop=(hc == HC - 1))
                    yield
                for s in range(2):
                    nc.vector.tensor_copy(
                        qt_sb[:, s, h, :], ps[:, s * 128:(s + 1) * 128])
                yield

            def o_proj(g, store, strips=(0, 1)):
                # stationary = ao q-block, rhs = Wo rows: out [q, d_out]
                for s in strips:
                    for oc in range(4):
                        fpt = fps.tile([128, 512], F32, tag="fp",
                                       name=f"op{g}_{s}_{oc}")
                        fp = fpt[:]
                        for hh in range(4):
                            nc.tensor.matmul(
                                fp,
                                ao_sb[:, g, hh, s * 128:(s + 1) * 128],
                                wos[g][:, hh * HID + oc * 512:
                                       hh * HID + (oc + 1) * 512],
                                start=(hh == 0), stop=(hh == 3))
                        osl = out_acc[:, s, oc * 512:(oc + 1) * 512]
                        if g == 0:
                            nc.vector.tensor_copy(osl, fp)
                        else:
                            nc.vector.tensor_add(osl, osl, fp)
                        if store:
                            nc.sync.dma_start(
                                outT.ap()[s * 128:(s + 1) * 128,
                                          oc * 512:(oc + 1) * 512], osl)
                        yield

            def chain(*gens):
                for gg in gens:
                    for _ in gg:
                        yield

            def strip_fin(g, s, acc2, accB):
                # per-strip epilogue: evacuate ao, build denominator via a
                # PE ones-matmul, normalize. Strip 0's key range ends at
                # kc=55, so its epilogue overlaps the masked-region chunks.
                nc.vector.tensor_copy(
                    ao_sb[:, g, :, s * 128:(s + 1) * 128],
                    ot_pss[g][:, s * 512:(s + 1) * 512]
                    .rearrange("p (h q) -> p h q", q=128))
                accF = attw.tile([128, 512], BF, tag="accf", bufs=2,
                                 name=f"accf{g}_{s}")
                nc.vector.tensor_add(accF[:], acc2[:, s * 512:(s + 1) * 512],
                                     acc2[:, 1024 + s * 512:1536 + s * 512])
                if s == 1:
                    nc.vector.tensor_add(accF[:], accF[:], accB[:])
                dent = fps.tile([128, 512], F32, tag="fp",
                                name=f"den{g}_{s}")
                den = dent[:]
                nc.tensor.matmul(den, ones_sb[:], accF[:],
                                 start=True, stop=True)
                recip_f = attw.tile([128, 512], F32, tag="recipf", bufs=2,
                                    name=f"rf{g}_{s}")
                nc.vector.reciprocal_approx_fast(recip_f[:], den)
                recip_b = attw.tile([128, 512], BF, tag="recipb", bufs=2,
                                    name=f"rb{g}_{s}")
                nc.vector.tensor_copy(recip_b[:], recip_f[:])
                nc.vector.tensor_mul(
                    ao_sb[:, g, :, s * 128:(s + 1) * 128],
                    ao_sb[:, g, :, s * 128:(s + 1) * 128],
                    recip_b[:].rearrange("p (h q) -> p h q", q=128))

            # all heads projected up front: this work hides the wait for
            # AGa, whose start is bounded by cross-core skew
            for h in range(H):
                for _ in q_head(h):
                    pass

            # filler schedules: previous group's o_proj
            fillers = {
                0: chain(),
                1: chain(o_proj(0, False)),
                2: chain(o_proj(1, False)),
                3: chain(o_proj(2, False)),
            }

            for g in range(HKV):
                wo_g = wop.tile([128, 4 * HID], BF, tag="wog",
                                name=f"wog{g}")
                nc.sync.dma_start(wo_g[:], woP.ap()[g])
                wos[g] = wo_g
                filler = fillers[g]

                ot_ps = otps.tile([128, 1024], F32, tag="ot")
                ot_pss[g] = ot_ps
                acc2 = accp.tile([128, 2048], BF, tag="acc")   # even|odd kc
                accB = accp.tile([128, 512], BF, tag="accb")   # kc>=56, s1
                ex2 = None
                pends = []   # software pipeline: PV trails S/exp by two kc

                def emit_pv(p):
                    kc_p, vs_p, l_p, ex_s0, ex_s1 = p
                    if ex_s0 is not None:
                        nc.tensor.matmul(
                            ot_ps[:, 0:512], vs_p[:, l_p, :], ex_s0,
                            start=(kc_p == 0), stop=(kc_p == 55))
                    nc.tensor.matmul(
                        ot_ps[:, 512:1024], vs_p[:, l_p, :], ex_s1,
                        start=(kc_p == 0), stop=(kc_p == KC - 1))

                for r in range(N_CORES):
                    kt_slab = kvstream.tile([128, LKV], BF, tag="kt")
                    if g == 0:
                        kfull = gathA[r, 0:KREG].rearrange("(p c) -> p c",
                                                           p=128)
                        for h2 in range(2):
                            nc.sync.dma_start(
                                kt_slab[:, h2 * 512:(h2 + 1) * 512],
                                kfull[:, h2 * 512:(h2 + 1) * 512])
                    else:
                        nc.sync.dma_start(
                            kt_slab[:],
                            gathK[g - 1][r].rearrange(
                                "(p a) c -> p (a c)", p=128))
                    v_slab = kvstream.tile([128, LKV // 128, D], BF, tag="v")
                    if g == 0:
                        vsrc = gathA[r, KREG:KREG + VREG]
                    else:
                        vsrc = gathB[r, (g - 1) * VREG:g * VREG]
                    nc.sync.dma_start(
                        v_slab[:],
                        vsrc.rearrange("(p a d) -> p a d", p=128, d=D))
                    for l in range(LKV // 128):
                        kc = r * (LKV // 128) + l
                        j = kc - BND
                        eps = kc & 1
                        ktc = kt_slab[:, l * 128:(l + 1) * 128]
                        st = stps.tile([128, 1024], F32, tag="st")
                        if kc < 56:
                            for s in range(2):
                                nc.tensor.matmul(
                                    st[:, s * 512:(s + 1) * 512], ktc,
                                    qt_sb[:, s, g * 4:(g + 1) * 4, :],
                                    start=True, stop=True)
                            if eps == 0:
                                ex2 = attw.tile([128, 2048], BF, tag="ex")
                            exh = ex2[:, eps * 1024:(eps + 1) * 1024]
                            nc.scalar.activation(
                                exh, st[:],
                                mybir.ActivationFunctionType.Exp, scale=SCALE)
                            if j >= 0:
                                # strip-0 boundary mask (ones rows harmless)
                                nc.vector.tensor_mul(
                                    ex2[:, eps * 1024:eps * 1024 + 512],
                                    ex2[:, eps * 1024:eps * 1024 + 512],
                                    mask_sb[:, j, :])
                            cur = (kc, v_slab, l,
                                   ex2[:, eps * 1024:eps * 1024 + 512],
                                   ex2[:, eps * 1024 + 512:(eps + 1) * 1024])
                            pends.append(cur)
                            if len(pends) > 3:
                                emit_pv(pends.pop(0))
                            if eps == 1:
                                if kc == 1:
                                    nc.vector.tensor_copy(acc2[:], ex2[:])
                                else:
                                    nc.vector.tensor_add(
                                        acc2[:], acc2[:], ex2[:])
                        else:
                            # strip 0 fully masked for every core: s1 only
                            nc.tensor.matmul(
                                st[:, 512:1024], ktc,
                                qt_sb[:, 1, g * 4:(g + 1) * 4, :],
                                start=True, stop=True)
                            exB = attw.tile([128, 512], BF, tag="exb", bufs=2)
                            nc.scalar.activation(
                                exB[:], st[:, 512:1024],
                                mybir.ActivationFunctionType.Exp, scale=SCALE)
                            nc.vector.tensor_mul(
                                exB[:], exB[:], mask_sb[:, j, :])
                            cur = (kc, v_slab, l, None, exB[:])
                            pends.append(cur)
                            if len(pends) > 3:
                                emit_pv(pends.pop(0))
                            if kc == 56:
                                nc.vector.tensor_copy(accB[:], exB[:])
                            else:
                                nc.vector.tensor_add(accB[:], accB[:], exB[:])
                        next(filler, None)
                        if kc == 58:
                            strip_fin(g, 0, acc2, accB)
                            if g == HKV - 1:
                                filler = o_proj(g, store=True, strips=(0,))
                for p in pends:
                    emit_pv(p)
                pends = []
                strip_fin(g, 1, acc2, accB)
            for _ in o_proj(HKV - 1, store=True, strips=(1,)):
                pass


def _get_nc():
    if "nc" not in _CACHE:
        _CACHE["nc"] = _build()
    return _CACHE["nc"]


def _make_in_maps(x_q, x_kv, Wq, Wk, Wv, Wo):
    xqT_full = np.ascontiguousarray(x_q[0].T)           # [HID, SL]
    xkvT_full = np.ascontiguousarray(x_kv[0].T)         # [HID, SKV]
    # partition-major packs (shared across cores)
    wqP = np.ascontiguousarray(
        Wq.reshape(H, D, HC, 128).transpose(3, 0, 2, 1)
    ).reshape(128, H, HC * D).astype(BF16)
    wkT = np.ascontiguousarray(
        Wk.T.reshape(HC, 128, HKV * D).transpose(1, 0, 2)).astype(BF16)
    wvT = np.ascontiguousarray(
        Wv.T.reshape(HC, 128, HKV * D).transpose(1, 0, 2)).astype(BF16)
    woP = np.ascontiguousarray(
        Wo.T.reshape(HKV, 4, 128, HID).transpose(0, 2, 1, 3)
    ).reshape(HKV, 128, 4 * HID).astype(BF16)

    in_maps = []
    kk = np.arange(128)
    for c in range(N_CORES):
        s0, s1 = c, 15 - c
        xqT = np.concatenate(
            [xqT_full[:, s0 * 128:(s0 + 1) * 128],
             xqT_full[:, s1 * 128:(s1 + 1) * 128]], axis=1)
        xqT = np.ascontiguousarray(
            xqT.reshape(HC, 128, QS).transpose(1, 0, 2)).astype(BF16)
        xkvT = np.ascontiguousarray(
            xkvT_full[:, c * LKV:(c + 1) * LKV]
            .reshape(HC, 128, LKV).transpose(1, 0, 2)).astype(BF16)
        # boundary masks: j<8 -> strip0 (=c), j>=8 -> strip1 (=15-c);
        # [128 keys, 128 q] tiled across the 4 heads of a group
        mask = np.zeros((16, 128, 128), dtype=np.float32)
        for jj in range(16):
            st_ = s0 if jj < 8 else s1
            key_g = (BND + jj) * 128 + kk
            q_g = RANK_OFF + st_ * 128 + kk
            mask[jj] = (key_g[:, None] <= q_g[None, :])
        mask4 = np.tile(mask, (1, 1, 4))                # [16, 128, 512]
        mask4 = np.ascontiguousarray(mask4.transpose(1, 0, 2)).astype(BF16)
        in_maps.append({
            "xqT": xqT, "xkvT": xkvT, "wqP": wqP, "wkT": wkT,
            "wvT": wvT, "woP": woP, "mask": mask4,
        })
    return in_maps


def _unshard(results):
    out = np.empty((1, SL, HID), dtype=np.float32)
    for c in range(N_CORES):
        outT = results[c]["outT"]                       # [QS, HID]
        s0, s1 = c, 15 - c
        out[0, s0 * 128:(s0 + 1) * 128, :] = outT[0:128]
        out[0, s1 * 128:(s1 + 1) * 128, :] = outT[128:256]
    return out


def kernel(x_q, x_kv, Wq, Wk, Wv, Wo, _trace=False, _result_box=None):
    nc = _get_nc()
    in_maps = _make_in_maps(x_q, x_kv, Wq, Wk, Wv, Wo)
    res = bass_utils.run_bass_kernel_spmd(
        nc, in_maps, core_ids=list(range(N_CORES)), trace=_trace)
    if _result_box is not None:
        _result_box.append(res)
    return _unshard(res.results)
